# revision 1
# baseline (speedup 1.0000x reference)
"""CNN+SE+LSTM fused Trainium2 kernel.

Data-parallel over batch: B=2048 split across 8 NeuronCores (256 each).
All matmuls run as fp32r (single-pass fp32, ~tf32 precision) on the PE;
the sigmoid/SE-scale/maxpool tensor path runs in bf16 on the DVE (2x mode).

Per-core pipeline (phase 1, per 8-batch "group", SE batched per block):
  conv1x1 (c^T layout [u, (b,w)]) -> sigmoid(+bias) on ACT (bf16 out) ->
  channel-mean via ones-matmul -> SE softmax (tiny matmuls + DRAM-bounce
  transposes) -> GpSimd cast-DMA broadcast -> bf16 scale + max-over-window
  on DVE -> pooled^T.
Phase 2: 2-layer bidirectional LSTM (single step, h0=c0=0 so the forget gate
is dead and the h@w_hh term vanishes) -> tanh classifier head -> [1, 256].

The final SE blocks are smaller so the tail SE chain doesn't gate the LSTM.
DMA routing: big x loads alternate the two HWDGE rings (SP/ACT); weights and
SE bounce DMAs ride GpSimd SWDGE; w1f prefetches during phase 1.
"""

import numpy as np

import concourse.bass as bass
import concourse.tile as tile
from concourse import bacc, mybir
from concourse.bass_utils import run_bass_kernel_spmd

B, W, D, U, H = 2048, 64, 512, 512, 512
NC = 8
BS = B // NC          # 256 batch rows per core
GB = 8                # batches per group (8 * W = 512 matmul columns)
NG = BS // GB         # 32 groups
BLOCKS = [4, 4, 4, 4, 4, 4, 2, 2, 2, 1, 1]   # SE batching; tapered tail
assert sum(BLOCKS) == NG
DC = D // 128         # 4 contraction chunks
UC = U // 128         # 4 output-channel chunks

dt = mybir.dt
AF = mybir.ActivationFunctionType
ALU = mybir.AluOpType
AX = mybir.AxisListType

_STATE = None


def _build_bass(unroll=1):
    nc = bacc.Bacc("TRN2", target_bir_lowering=False, debug=False,
                   num_devices=NC, num_swdge_queues=4)

    f32, f32r, bf16 = dt.float32, dt.float32r, dt.bfloat16

    d_xt = nc.dram_tensor("xt", [D, BS, W], f32r, kind="ExternalInput").ap()
    d_cw = nc.dram_tensor("cw", [128, DC * U], f32r, kind="ExternalInput").ap()
    d_cb = nc.dram_tensor("cb", [128, UC], f32, kind="ExternalInput").ap()
    d_ones = nc.dram_tensor("onescol", [128, 1], bf16, kind="ExternalInput").ap()
    d_ones32 = nc.dram_tensor("ones32", [1, 4 * GB], f32r, kind="ExternalInput").ap()
    d_sewt = nc.dram_tensor("sewt", [W, W], f32r, kind="ExternalInput").ap()
    d_seb = nc.dram_tensor("seb", [1, W], f32r, kind="ExternalInput").ap()
    d_w0, d_b0, d_w1, d_b1 = {}, {}, {}, {}
    for s in ("f", "r"):
        d_w0[s] = nc.dram_tensor(f"w0{s}", [128, 4 * 1536], f32r, kind="ExternalInput").ap()
        d_b0[s] = nc.dram_tensor(f"b0{s}", [128, 12], f32, kind="ExternalInput").ap()
        d_w1[s] = nc.dram_tensor(f"w1{s}", [128, 8 * 1536], f32r, kind="ExternalInput").ap()
        d_b1[s] = nc.dram_tensor(f"b1{s}", [128, 12], f32, kind="ExternalInput").ap()
    d_clsw = nc.dram_tensor("clsw", [128, 8], f32r, kind="ExternalInput").ap()
    d_clsb = nc.dram_tensor("clsb", [1, 1], f32, kind="ExternalInput").ap()
    d_out = nc.dram_tensor("out", [1, BS], f32, kind="ExternalOutput").ap()

    with tile.TileContext(nc) as tc:
        with tc.tile_pool(name="wpool", bufs=1) as wpool, \
             tc.tile_pool(name="persist", bufs=1) as persist:
            # static weights, staged up front on the SWDGE path
            cw_t = wpool.tile([128, DC * U], f32r, name="cw_t")
            nc.gpsimd.dma_start(cw_t[:], d_cw)
            cb_t = wpool.tile([128, UC], f32, name="cb_t")
            nc.gpsimd.dma_start(cb_t[:], d_cb)
            ones_t = wpool.tile([128, 1], bf16, name="ones_t")
            nc.gpsimd.dma_start(ones_t[:], d_ones)
            ones32_t = wpool.tile([1, 4 * GB], f32r, name="ones32_t")
            nc.gpsimd.dma_start(ones32_t[:], d_ones32)
            sewt_t = wpool.tile([W, W], f32r, name="sewt_t")
            nc.gpsimd.dma_start(sewt_t[:], d_sewt)
            seb_t = wpool.tile([1, W], f32r, name="seb_t")
            nc.gpsimd.dma_start(seb_t[:], d_seb)
            w0_t, b0_t, b1_t = {}, {}, {}
            for s in ("f", "r"):
                w0_t[s] = wpool.tile([128, 4 * 1536], f32r, name=f"w0{s}_t")
                nc.gpsimd.dma_start(w0_t[s][:], d_w0[s])
                b0_t[s] = wpool.tile([128, 12], f32, name=f"b0{s}_t")
                nc.gpsimd.dma_start(b0_t[s][:], d_b0[s])
                b1_t[s] = wpool.tile([128, 12], f32, name=f"b1{s}_t")
                nc.gpsimd.dma_start(b1_t[s][:], d_b1[s])
            clsw_t = wpool.tile([128, 8], f32r, name="clsw_t")
            nc.gpsimd.dma_start(clsw_t[:], d_clsw)
            clsb_t = wpool.tile([1, 1], f32, name="clsb_t")
            nc.gpsimd.dma_start(clsb_t[:], d_clsb)

            # pooled^T accumulator [128, uc, BS], filled per group
            pooledT = persist.tile([128, UC, BS], f32r, name="pooledT")

            # l1 forward weights: prefetched during phase 1 (SWDGE path)
            w1_t = {}
            w1_t["f"] = persist.tile([128, 8 * 1536], f32r, name="w1f_t")
            nc.gpsimd.dma_start(w1_t["f"][:], d_w1["f"])

            for _rep in range(unroll):
                # ---------------- phase 1: conv + SE + maxpool ----------------
                with tc.tile_pool(name="xp", bufs=3) as xp, \
                     tc.tile_pool(name="sigp", bufs=10) as sigp, \
                     tc.tile_pool(name="scp", bufs=2) as scp, \
                     tc.tile_pool(name="bcp", bufs=2) as bcp, \
                     tc.tile_pool(name="sep", bufs=3) as sep, \
                     tc.tile_pool(name="drp", bufs=4, space="DRAM") as drp, \
                     tc.tile_pool(name="cps", bufs=6, space="PSUM") as cps, \
                     tc.tile_pool(name="usps", bufs=1, space="PSUM") as usps, \
                     tc.tile_pool(name="lgps", bufs=1, space="PSUM") as lgps:
                    g0 = 0
                    for nblk in BLOCKS:
                        gs = list(range(g0, g0 + nblk))
                        g0 += nblk
                        scr1 = drp.tile([4, GB * W], f32r, name="scr1", tag="scr1")
                        sig_blk = []
                        for gi, g in enumerate(gs):
                            xt = xp.tile([128, DC, GB * W], f32r, name="xt", tag="x")
                            # one 512 KiB DMA per group, alternating HWDGE rings
                            dma_eng = nc.sync if g % 2 == 0 else nc.scalar
                            src = d_xt[:, g * GB:(g + 1) * GB, :].rearrange(
                                "(dc p) b w -> p dc (b w)", p=128
                            )
                            dma_eng.dma_start(xt[:], src)

                            us = usps.tile([1, GB * W], f32, name="us", tag="us")
                            sigg = sigp.tile([128, UC, GB * W], bf16, name="sigg", tag="sig")
                            sig_blk.append(sigg)
                            for uc in range(UC):
                                cp = cps.tile([128, GB * W], f32, name="cp", tag="cp")
                                for dc in range(DC):
                                    nc.tensor.matmul(
                                        cp[:],
                                        cw_t[:, dc * U + uc * 128: dc * U + (uc + 1) * 128],
                                        xt[:, dc, :],
                                        start=(dc == 0),
                                        stop=(dc == DC - 1),
                                    )
                                nc.scalar.activation(
                                    sigg[:, uc, :], cp[:], AF.Sigmoid,
                                    bias=cb_t[:, uc:uc + 1], scale=1.0,
                                )
                                nc.tensor.matmul(
                                    us[:], ones_t[:], sigg[:, uc, :],
                                    start=(uc == 0), stop=(uc == UC - 1),
                                )
                            # avg row -> DRAM scratch (DVE copy keeps ACT on Sigmoid)
                            avg_row = sep.tile([1, GB * W], f32r, name="avg_row", tag="avgrow")
                            nc.vector.tensor_copy(avg_row[:], us[:])
                            nc.sync.dma_start(scr1[gi:gi + 1, :], avg_row[:])

                        # SE for the whole block: avgT [w, (gi b)]
                        nb = len(gs) * GB
                        avgT = sep.tile([W, 4 * GB], f32r, name="avgT", tag="avgT")
                        nc.sync.dma_start(
                            avgT[:, 0:nb],
                            scr1[0:len(gs), :].rearrange("g (b w) -> (w) g b", w=W),
                        )
                        lg = lgps.tile([4 * GB, W], f32, name="lg", tag="lg")
                        nc.tensor.matmul(lg[0:nb, :], avgT[:, 0:nb], sewt_t[:],
                                         start=True, stop=False)
                        nc.tensor.matmul(lg[0:nb, :], ones32_t[:, 0:nb], seb_t[:],
                                         start=False, stop=True)
                        E = sep.tile([4 * GB, W], f32, name="E", tag="E")
                        nc.scalar.activation(E[0:nb, :], lg[0:nb, :], AF.Exp)
                        S = sep.tile([4 * GB, 1], f32, name="S", tag="S")
                        nc.vector.reduce_sum(S[0:nb, :], E[0:nb, :], axis=AX.X)
                        R = sep.tile([4 * GB, 1], f32, name="R", tag="R")
                        nc.vector.reciprocal(R[0:nb, :], S[0:nb, :])
                        seg = sep.tile([4 * GB, W], f32r, name="seg", tag="seg")
                        nc.vector.tensor_scalar_mul(seg[0:nb, :], E[0:nb, :], R[0:nb, 0:1])
                        scr2 = drp.tile([4 * GB, W], f32r, name="scr2", tag="scr2")
                        nc.scalar.dma_start(scr2[0:nb, :], seg[0:nb, :])
                        # broadcast to all partitions with f32r->bf16 cast (SWDGE)
                        sebc = bcp.tile([128, 4 * GB * W], bf16, name="sebc", tag="sebc")
                        nc.gpsimd.dma_start(
                            sebc[:, 0:nb * W],
                            scr2[0:nb, :].bitcast(f32)
                            .rearrange("b w -> (b w)").unsqueeze(0)
                            .broadcast_to([128, nb * W]),
                        )
                        for gi, g in enumerate(gs):
                            scaled = scp.tile([128, UC, GB * W], bf16, name="scaled", tag="scaled")
                            nc.vector.tensor_mul(
                                scaled[:],
                                sig_blk[gi][:],
                                sebc[:, gi * GB * W:(gi + 1) * GB * W]
                                .unsqueeze(1).broadcast_to([128, UC, GB * W]),
                            )
                            pbf = scp.tile([128, UC * GB], bf16, name="pbf", tag="pbf")
                            nc.vector.tensor_reduce(
                                pbf[:],
                                scaled[:].rearrange("p u (b w) -> p (u b) w", w=W),
                                axis=AX.X,
                                op=ALU.max,
                            )
                            nc.vector.tensor_copy(
                                pooledT[:, :, g * GB:(g + 1) * GB],
                                pbf[:].rearrange("p (u b) -> p u b", u=UC),
                            )

                # ---------------- phase 2: LSTM + classifier ----------------
                with tc.tile_pool(name="w1rp", bufs=1) as w1rp, \
                     tc.tile_pool(name="lp", bufs=2) as lp, \
                     tc.tile_pool(name="op", bufs=1) as op, \
                     tc.tile_pool(name="gps", bufs=6, space="PSUM") as gps, \
                     tc.tile_pool(name="clsps", bufs=1, space="PSUM") as clsps:
                    w1_t["r"] = w1rp.tile([128, 8 * 1536], f32r, name="w1r_t")
                    nc.gpsimd.dma_start(w1_t["r"][:], d_w1["r"])

                    def lstm_dir(w_t, b_t, kcs, rhs_tiles, out_tiles, out_tanh):
                        # i/o gates first (Sigmoid run), then g + tanh(c) (Tanh run)
                        gate_sb = {}
                        for gi, func in ((0, AF.Sigmoid), (2, AF.Sigmoid), (1, AF.Tanh)):
                            for q in range(4):
                                m = gi * 4 + q
                                gp = gps.tile([128, BS], f32, name="gp", tag="gp")
                                for kc in range(kcs):
                                    nc.tensor.matmul(
                                        gp[:],
                                        w_t[:, kc * 1536 + m * 128: kc * 1536 + (m + 1) * 128],
                                        rhs_tiles[kc],
                                        start=(kc == 0),
                                        stop=(kc == kcs - 1),
                                    )
                                gs_ = lp.tile([128, BS], f32, name="gs", tag=f"gate{gi}q{q}")
                                nc.scalar.activation(
                                    gs_[:], gp[:], func, bias=b_t[:, m:m + 1], scale=1.0
                                )
                                gate_sb[(gi, q)] = gs_
                        for q in range(4):
                            cpre = lp.tile([128, BS], f32, name="cpre", tag="cpre")
                            nc.vector.tensor_mul(cpre[:], gate_sb[(0, q)][:], gate_sb[(1, q)][:])
                            tcl = lp.tile([128, BS], f32, name="tcl", tag="tcl")
                            nc.scalar.activation(tcl[:], cpre[:], AF.Tanh)
                            if out_tanh:
                                h = lp.tile([128, BS], f32, name="h", tag="h")
                                nc.vector.tensor_mul(h[:], gate_sb[(2, q)][:], tcl[:])
                                nc.scalar.activation(out_tiles[q], h[:], AF.Tanh)
                            else:
                                nc.vector.tensor_mul(out_tiles[q], gate_sb[(2, q)][:], tcl[:])

                    o0T = [op.tile([128, BS], f32r, name=f"o0T{i}")[:] for i in range(8)]
                    o1T = [op.tile([128, BS], f32r, name=f"o1T{i}")[:] for i in range(8)]
                    pooled_rhs = [pooledT[:, kc, :] for kc in range(UC)]
                    lstm_dir(w0_t["f"], b0_t["f"], 4, pooled_rhs, o0T[0:4], False)
                    lstm_dir(w0_t["r"], b0_t["r"], 4, pooled_rhs, o0T[4:8], False)
                    lstm_dir(w1_t["f"], b1_t["f"], 8, o0T, o1T[0:4], True)
                    lstm_dir(w1_t["r"], b1_t["r"], 8, o0T, o1T[4:8], True)

                    clsp = clsps.tile([1, BS], f32, name="clsp")
                    for kc in range(8):
                        nc.tensor.matmul(
                            clsp[:], clsw_t[:, kc:kc + 1], o1T[kc],
                            start=(kc == 0), stop=(kc == 7),
                        )
                    outsb = lp.tile([1, BS], f32, name="outsb", tag="outsb")
                    nc.scalar.activation(
                        outsb[:], clsp[:], AF.Tanh, bias=clsb_t[0:1, 0:1], scale=1.0
                    )
                    nc.sync.dma_start(d_out, outsb[:])

    nc.compile()
    return nc


def _prep_weights(i):
    """Host-side packing of the replicated (non-batch) tensors."""
    import ml_dtypes

    def f32(a):
        return np.ascontiguousarray(a, dtype=np.float32)

    out = {}
    out["cw"] = f32(i["conv_w"].T.reshape(DC, 128, U).transpose(1, 0, 2).reshape(128, DC * U))
    out["cb"] = f32(i["conv_b"].reshape(UC, 128).T)
    out["onescol"] = np.full((128, 1), 1.0 / U, ml_dtypes.bfloat16)
    out["ones32"] = np.ones((1, 4 * GB), np.float32)
    out["sewt"] = f32(i["se_w"].T)
    out["seb"] = f32(i["se_b"].reshape(1, W))
    igo = np.r_[0:512, 1024:2048]  # drop dead forget gate
    for s, tag in (("f", "l0f"), ("r", "l0r")):
        wT = f32(i[f"w_ih_{tag}"]).T[:, igo]                      # [512, 1536]
        out[f"w0{s}"] = f32(wT.reshape(4, 128, 1536).transpose(1, 0, 2).reshape(128, 4 * 1536))
        bs = (f32(i[f"b_ih_{tag}"]) + f32(i[f"b_hh_{tag}"]))[igo]  # [1536]
        out[f"b0{s}"] = f32(bs.reshape(12, 128).T)
    for s, tag in (("f", "l1f"), ("r", "l1r")):
        wT = f32(i[f"w_ih_{tag}"]).T[:, igo]                      # [1024, 1536]
        out[f"w1{s}"] = f32(wT.reshape(8, 128, 1536).transpose(1, 0, 2).reshape(128, 8 * 1536))
        bs = (f32(i[f"b_ih_{tag}"]) + f32(i[f"b_hh_{tag}"]))[igo]
        out[f"b1{s}"] = f32(bs.reshape(12, 128).T)
    out["clsw"] = f32(i["cls_w"].reshape(2 * H)).reshape(8, 128).T.copy()
    out["clsb"] = f32(i["cls_b"]).reshape(1, 1)
    return out


def _get_nc():
    global _STATE
    if _STATE is None:
        _STATE = _build_bass()
    return _STATE


def make_in_maps(**inputs):
    w = _prep_weights(inputs)
    xt = np.ascontiguousarray(
        np.asarray(inputs["x"], dtype=np.float32).transpose(2, 0, 1)
    )  # [D, B, W]
    maps = []
    for c in range(NC):
        m = dict(w)
        m["xt"] = np.ascontiguousarray(xt[:, c * BS:(c + 1) * BS, :])
        maps.append(m)
    return maps


def kernel(**inputs):
    nc = _get_nc()
    maps = make_in_maps(**inputs)
    res = run_bass_kernel_spmd(nc, maps, core_ids=list(range(NC)))
    out = np.empty((B, 1), np.float32)
    for c in range(NC):
        out[c * BS:(c + 1) * BS, 0] = res.results[c]["out"][0]
    return out



# revision 42
# speedup vs baseline: 1.0409x; 1.0409x over previous
"""CNN+SE+LSTM fused Trainium2 kernel (v2).

Data-parallel over batch: B=2048 split across 8 NeuronCores (256 each).

Key techniques vs v1:
  - conv1x1 runs in fp8e4m3 with MatmulPerfMode.DoubleRow (2 packed K
    values/cell): 2x PE throughput and 4x less x DMA. conv_w is scaled by
    32 host-side so all weights are fp8-normal; 1/32 folds into the
    sigmoid's scale operand.
  - sigmoid is applied per (uc, group-pair) on a 2-bank PSUM tile
    ([128, 2, 512]) to amortize ACT fixed overhead; channel-mean rows for
    the 4 groups of an SE block land at PSUM partitions 0/32/64/96 of one
    bank (PE column tiling) so one DMA ships the whole block's avg.
  - SE softmax avoids the Exp activation table entirely (Sigmoid and Tanh
    share an ACT table set, Exp does not): e^z = (1+tanh(z/2))/(1-tanh(z/2)),
    with DVE reciprocal_approx_fast. Zero LoadActFuncSet swaps mid-kernel.
  - maxpool-over-window runs as a binary tensor_tensor(max) tree on the DVE
    in bf16 (2x mode), ~1.6x faster than the 1x-mode tensor_reduce.
  - the 2-layer bidirectional LSTM is processed in 3 batch chunks
    (128/96/32 cols) interleaved into the conv/SE stream so its matmuls fill
    PE idle time and the tail after the last pooled group is short. Gate
    biases are added via tiny k=1 bias-matmuls so gate activations can be
    PSUM-func-grouped ([128, 4, cols] per i/g/o); gate element-wise products
    run on GpSimd (Pool) to keep the DVE free.
"""

import numpy as np

import concourse.bass as bass
import concourse.tile as tile
from concourse import bacc, mybir
from concourse.bass_utils import run_bass_kernel_spmd

B, W, D, U, H = 2048, 64, 512, 512, 512
NC = 8
BS = B // NC          # 256 batch rows per core
GB = 8                # batches per group (8 * W = 512 matmul columns)
NG = BS // GB         # 32 groups
GBW = GB * W
BLOCKS = [2, 2, 4, 4, 4, 4, 4, 4, 2, 2]   # SE batching; cum 2,4,8..28,30,32
assert sum(BLOCKS) == NG
DC = D // 128         # 4 contraction chunks
UC = U // 128         # 4 output-channel chunks
# LSTM batch chunks in groups: [start_g, end_g)
CHUNKS = [(0, 16), (16, 28), (28, 32)]

dt = mybir.dt
AF = mybir.ActivationFunctionType
ALU = mybir.AluOpType
AX = mybir.AxisListType
DR = mybir.MatmulPerfMode.DoubleRow

_STATE = None


def _build_bass(unroll=1):
    nc = bacc.Bacc("TRN2", target_bir_lowering=False, debug=False,
                   num_devices=NC, num_swdge_queues=4)

    f32, f32r, bf16, f8 = dt.float32, dt.float32r, dt.bfloat16, dt.float8e4

    d_xt = nc.dram_tensor("xt", [D, BS, W], f8, kind="ExternalInput").ap()
    d_cw = nc.dram_tensor("cw", [128, DC * U], f8, kind="ExternalInput").ap()
    d_cb = nc.dram_tensor("cb", [128, UC], f32, kind="ExternalInput").ap()
    d_onesm = nc.dram_tensor("onesm", [128, 1], bf16, kind="ExternalInput").ap()
    d_ones32 = nc.dram_tensor("ones32", [1, 4 * GB], f32r, kind="ExternalInput").ap()
    d_onesc = nc.dram_tensor("onesc", [1, BS], bf16, kind="ExternalInput").ap()
    d_sewt = nc.dram_tensor("sewt", [W, W], f32r, kind="ExternalInput").ap()
    d_seb = nc.dram_tensor("seb", [1, W], f32r, kind="ExternalInput").ap()
    d_w0, d_bv0, d_w1, d_bv1 = {}, {}, {}, {}
    for s in ("f", "r"):
        d_w0[s] = nc.dram_tensor(f"w0{s}", [128, 4 * 1536], f8, kind="ExternalInput").ap()
        d_bv0[s] = nc.dram_tensor(f"bv0{s}", [1, 1536], bf16, kind="ExternalInput").ap()
        d_w1[s] = nc.dram_tensor(f"w1{s}", [128, 8 * 1536], f8, kind="ExternalInput").ap()
        d_bv1[s] = nc.dram_tensor(f"bv1{s}", [1, 1536], bf16, kind="ExternalInput").ap()
    d_clsw = nc.dram_tensor("clsw", [128, 8], bf16, kind="ExternalInput").ap()
    d_clsb = nc.dram_tensor("clsb", [1, 1], f32, kind="ExternalInput").ap()
    d_out = nc.dram_tensor("out", [1, BS], f32, kind="ExternalOutput").ap()

    with tile.TileContext(nc) as tc:
        with tc.tile_pool(name="wpool", bufs=1) as wpool, \
             tc.tile_pool(name="persist", bufs=1) as persist:
            # static weights, staged up front on the SWDGE path
            cw_t = wpool.tile([128, DC * U], f8, name="cw_t")
            nc.gpsimd.dma_start(cw_t[:], d_cw)
            cb_t = wpool.tile([128, UC], f32, name="cb_t")
            nc.gpsimd.dma_start(cb_t[:], d_cb)
            onesm_t = wpool.tile([128, 1], bf16, name="onesm_t")
            nc.gpsimd.dma_start(onesm_t[:], d_onesm)
            ones32_t = wpool.tile([1, 4 * GB], f32r, name="ones32_t")
            nc.gpsimd.dma_start(ones32_t[:], d_ones32)
            onesc_t = wpool.tile([1, BS], bf16, name="onesc_t")
            nc.gpsimd.dma_start(onesc_t[:], d_onesc)
            sewt_t = wpool.tile([W, W], f32r, name="sewt_t")
            nc.gpsimd.dma_start(sewt_t[:], d_sewt)
            seb_t = wpool.tile([1, W], f32r, name="seb_t")
            nc.gpsimd.dma_start(seb_t[:], d_seb)
            # LSTM weight tiles are allocated here but their (re)loads are
            # issued inside each rep at block 1/3 so the startup DMA slots
            # belong to the x loads.
            w0_t, bv0_t, w1_t, bv1_t = {}, {}, {}, {}
            for s in ("f", "r"):
                w0_t[s] = wpool.tile([128, 4 * 1536], f8, name=f"w0{s}_t")
                bv0_t[s] = wpool.tile([1, 1536], bf16, name=f"bv0{s}_t")
                w1_t[s] = wpool.tile([128, 8 * 1536], f8, name=f"w1{s}_t")
                bv1_t[s] = wpool.tile([1, 1536], bf16, name=f"bv1{s}_t")
            clsw_t = wpool.tile([128, 8], bf16, name="clsw_t")
            nc.gpsimd.dma_start(clsw_t[:], d_clsw)
            clsb_t = wpool.tile([1, 1], f32, name="clsb_t")
            nc.gpsimd.dma_start(clsb_t[:], d_clsb)

            cw_r = cw_t[:].rearrange("p (dc u) -> p dc u", dc=DC)

            pooledT = persist.tile([128, UC, BS], bf16, name="pooledT")
            o0T = persist.tile([128, 8, BS], bf16, name="o0T")
            outsb = persist.tile([1, BS], f32, name="outsb")

            for _rep in range(unroll):
                with tc.tile_pool(name="xp", bufs=4) as xp, \
                     tc.tile_pool(name="sigp", bufs=8) as sigp, \
                     tc.tile_pool(name="scp", bufs=3) as scp, \
                     tc.tile_pool(name="bcp", bufs=3) as bcp, \
                     tc.tile_pool(name="sep", bufs=3) as sep, \
                     tc.tile_pool(name="lp", bufs=2) as lp, \
                     tc.tile_pool(name="drp", bufs=4, space="DRAM") as drp, \
                     tc.tile_pool(name="pps", bufs=2, space="PSUM") as pps:
                    # PSUM budget (8 banks): tag "big" 2x4KB shared by conv cp
                    # and lstm gp (they alternate in time), "us" 2x2KB mean
                    # rows, "small" 2x2KB shared by SE lg and cls psum.

                    # ---------- LSTM emit helpers (interleaved) ----------
                    def lstm_layer(w_t, bv_t, kcs, rhs_fn, cg0, cg1, out_sl,
                                   out_tanh):
                        c0 = cg0 * GB
                        cols = (cg1 - cg0) * GB
                        gates = {}
                        for fi, func in ((0, AF.Sigmoid), (1, AF.Tanh),
                                         (2, AF.Sigmoid)):
                            gp = pps.tile([128, 4, cols], f32, name="gp", tag="big")
                            for q in range(4):
                                m = fi * 4 + q
                                for kc in range(kcs):
                                    nc.tensor.matmul(
                                        gp[:, q, :],
                                        w_t[:, kc * 1536 + m * 128:
                                            kc * 1536 + (m + 1) * 128],
                                        rhs_fn(kc),
                                        start=(kc == 0), stop=False,
                                    )
                                nc.tensor.matmul(
                                    gp[:, q, :],
                                    bv_t[0:1, m * 128:(m + 1) * 128],
                                    onesc_t[0:1, 0:cols],
                                    start=False, stop=True,
                                )
                            gg = lp.tile([128, 4, cols], bf16, name="gg",
                                         tag=f"g{fi}")
                            nc.scalar.activation(gg[:], gp[:], func,
                                                 scale=1.0 / 32.0)
                            gates[fi] = gg
                        cpre = lp.tile([128, 4, cols], bf16, name="cpre", tag="cpre")
                        nc.gpsimd.tensor_mul(cpre[:], gates[0][:], gates[1][:])
                        tcl = lp.tile([128, 4, cols], bf16, name="tcl", tag="tcl")
                        nc.scalar.activation(tcl[:], cpre[:], AF.Tanh)
                        if out_tanh:
                            h = lp.tile([128, 4, cols], bf16, name="h", tag="h")
                            nc.gpsimd.tensor_mul(h[:], gates[2][:], tcl[:])
                            nc.scalar.activation(out_sl, h[:], AF.Tanh)
                        else:
                            nc.gpsimd.tensor_mul(out_sl, gates[2][:], tcl[:])

                    o1c = {}

                    def emit_l0(ci):
                        cg0, cg1 = CHUNKS[ci]
                        c0 = cg0 * GB
                        cols = (cg1 - cg0) * GB
                        for si, s in enumerate(("f", "r")):
                            lstm_layer(
                                w0_t[s], bv0_t[s], 4,
                                lambda kc: pooledT[:, kc, c0:c0 + cols],
                                cg0, cg1,
                                o0T[:, 4 * si:4 * si + 4, c0:c0 + cols],
                                False,
                            )

                    def emit_l1(ci):
                        cg0, cg1 = CHUNKS[ci]
                        c0 = cg0 * GB
                        cols = (cg1 - cg0) * GB
                        oc = lp.tile([128, 8, cols], bf16, name="o1c", tag="o1c")
                        o1c[ci] = oc
                        for si, s in enumerate(("f", "r")):
                            lstm_layer(
                                w1_t[s], bv1_t[s], 8,
                                lambda kc: o0T[:, kc, c0:c0 + cols],
                                cg0, cg1,
                                oc[:, 4 * si:4 * si + 4, :],
                                True,
                            )

                    def emit_cls(ci):
                        cg0, cg1 = CHUNKS[ci]
                        c0 = cg0 * GB
                        cols = (cg1 - cg0) * GB
                        oc = o1c[ci]
                        clsp = pps.tile([1, cols], f32, name="clsp", tag="small")
                        for kc in range(8):
                            nc.tensor.matmul(
                                clsp[:], clsw_t[:, kc:kc + 1], oc[:, kc, :],
                                start=(kc == 0), stop=(kc == 7),
                            )
                        nc.scalar.activation(
                            outsb[0:1, c0:c0 + cols], clsp[:], AF.Tanh,
                            bias=clsb_t[0:1, 0:1], scale=1.0,
                        )

                    # keys are block indices; pooled for blocks <= b-1 is
                    # complete after block b's pending_scale emission.
                    emit_after = {
                        6: [lambda: emit_l0(0)],          # pooled g0..15 (b0-4)
                        7: [lambda: emit_l1(0), lambda: emit_cls(0)],
                        9: [lambda: emit_l0(1)],          # pooled g16..27 (b5-7)
                        "flush": [lambda: emit_l1(1), lambda: emit_cls(1),
                                  lambda: emit_l0(2), lambda: emit_l1(2),
                                  lambda: emit_cls(2)],
                    }

                    # ---------- conv + SE + maxpool stream ----------
                    # scale/maxpool for block b-1 is emitted during block b so
                    # the DVE has work while block b's SE round-trip resolves.
                    g0 = 0
                    pending_scale = []
                    for bi, nblk in enumerate(BLOCKS):
                        if bi == 4:
                            # wait_until keeps the scheduler from hoisting
                            # these dep-free loads into the startup DMA burst;
                            # chunked so x loads interleave between slices.
                            # SP ring: no waits, so no head-of-line risk.
                            for ci_, s in enumerate(("f", "r")):
                                for kc in range(2):
                                    with tc.tile_wait_until(0.020 + 0.003 * (2 * ci_ + kc)):
                                        nc.gpsimd.dma_start(
                                            w0_t[s][:, kc * 3072:(kc + 1) * 3072],
                                            d_w0[s][:, kc * 3072:(kc + 1) * 3072])
                                with tc.tile_wait_until(0.028):
                                    nc.gpsimd.dma_start(bv0_t[s][:], d_bv0[s])
                        elif bi == 5:
                            for ci_, s in enumerate(("f", "r")):
                                for kc in range(4):
                                    with tc.tile_wait_until(0.034 + 0.003 * (4 * ci_ + kc)):
                                        nc.gpsimd.dma_start(
                                            w1_t[s][:, kc * 3072:(kc + 1) * 3072],
                                            d_w1[s][:, kc * 3072:(kc + 1) * 3072])
                                with tc.tile_wait_until(0.044):
                                    nc.gpsimd.dma_start(bv1_t[s][:], d_bv1[s])
                        gs = list(range(g0, g0 + nblk))
                        g0 += nblk
                        nb = nblk * GB
                        scr1 = drp.tile([4, GBW], f32r, name="scr1", tag="scr1")
                        sig_tiles = []
                        for ps_ in range(0, nblk, 2):
                            gpair = gs[ps_:ps_ + 2]
                            sigg = sigp.tile([128, UC, 2, GBW], bf16,
                                             name="sigg", tag="sig")
                            sig_tiles.append(sigg)
                            xts = []
                            for g in gpair:
                                xt = xp.tile([128, DC, GBW], f8, name="xt", tag="x")
                                nc.sync.dma_start(
                                    xt[:],
                                    d_xt[:, g * GB:(g + 1) * GB, :].rearrange(
                                        "(dc p) b w -> p dc (b w)", p=128),
                                )
                                xts.append(xt)
                            for uc in range(UC):
                                cp = pps.tile([128, 2, GBW], f32, name="cp", tag="big")
                                for gl in range(2):
                                    for kp in range(2):
                                        nc.tensor.matmul(
                                            cp[:, gl, :],
                                            cw_r[:, 2 * kp:2 * kp + 2,
                                                 uc * 128:(uc + 1) * 128],
                                            xts[gl][:, 2 * kp:2 * kp + 2, :],
                                            start=(kp == 0), stop=(kp == 1),
                                            perf_mode=DR,
                                        )
                                nc.scalar.activation(
                                    sigg[:, uc, :, :], cp[:], AF.Sigmoid,
                                    bias=cb_t[:, uc:uc + 1], scale=1.0 / 32.0,
                                )
                            # channel-mean rows for the pair at PSUM
                            # partitions 0 and 32 of a 1-bank tile
                            us2 = pps.tile([128, GBW], f32, name="us2", tag="us")
                            for gl, g in enumerate(gpair):
                                for uc in range(UC):
                                    nc.tensor.matmul(
                                        us2[32 * gl:32 * gl + 1, :],
                                        onesm_t[:],
                                        sigg[:, uc, gl, :],
                                        start=(uc == 0), stop=(uc == UC - 1),
                                    )
                            # Neither DMA nor GpSimd can read PSUM: bounce via
                            # a copy (alternating ACT/DVE to share the cost),
                            # then transpose straight into avgT (SBUF->SBUF)
                            # engines need partition step 1, so copy the whole
                            # 0..32 lane range (junk lanes 1..31 cost nothing:
                            # engine time scales with free size only), then
                            # bounce the two rows through DRAM for the
                            # w-transpose (DRAM APs have no partition rules)
                            avg2 = sep.tile([33, GBW], f32r, name="avg2", tag="avg2")
                            if (bi + ps_) % 2 == 0:
                                nc.scalar.copy(avg2[0:33, :], us2[0:33, :])
                            else:
                                nc.vector.tensor_copy(avg2[0:33, :], us2[0:33, :])
                            for gl in range(2):
                                nc.sync.dma_start(
                                    scr1[ps_ + gl:ps_ + gl + 1, :],
                                    avg2[32 * gl:32 * gl + 1, :],
                                )

                        # ---- SE for the block ----
                        avgT = sep.tile([W, 4 * GB], f32r, name="avgT", tag="avgT")
                        nc.sync.dma_start(
                            avgT[:, 0:nb],
                            scr1[0:nblk, :].rearrange("g (b w) -> (w) g b", w=W),
                        )
                        lg = pps.tile([4 * GB, W], f32, name="lg", tag="small")
                        nc.tensor.matmul(lg[0:nb, :], avgT[:, 0:nb], sewt_t[:],
                                         start=True, stop=False)
                        nc.tensor.matmul(lg[0:nb, :], ones32_t[:, 0:nb], seb_t[:],
                                         start=False, stop=True)
                        # softmax via tanh: e^z = (1+tanh(z/2))/(1-tanh(z/2))
                        th = sep.tile([4 * GB, W], f32, name="th", tag="th")
                        nc.scalar.activation(th[0:nb, :], lg[0:nb, :], AF.Tanh,
                                             scale=0.5)
                        den = sep.tile([4 * GB, W], f32, name="den", tag="den")
                        nc.vector.tensor_scalar(den[0:nb, :], th[0:nb, :],
                                                -1.0, 1.0, ALU.mult, ALU.add)
                        rden = sep.tile([4 * GB, W], f32, name="rden", tag="rden")
                        nc.vector.reciprocal_approx_fast(rden[0:nb, :], den[0:nb, :])
                        num = sep.tile([4 * GB, W], f32, name="num", tag="num")
                        nc.vector.tensor_scalar_add(num[0:nb, :], th[0:nb, :], 1.0)
                        E = sep.tile([4 * GB, W], f32, name="E", tag="E")
                        nc.vector.tensor_mul(E[0:nb, :], num[0:nb, :], rden[0:nb, :])
                        S = sep.tile([4 * GB, 1], f32, name="S", tag="S")
                        nc.vector.reduce_sum(S[0:nb, :], E[0:nb, :], axis=AX.X)
                        R = sep.tile([4 * GB, 1], f32, name="R", tag="R")
                        nc.vector.reciprocal_approx_fast(R[0:nb, :], S[0:nb, :])
                        seg = sep.tile([4 * GB, W], f32r, name="seg", tag="seg")
                        nc.vector.tensor_scalar_mul(seg[0:nb, :], E[0:nb, :],
                                                    R[0:nb, 0:1])
                        # scr2 on the SWDGE path: its seg-wait merges with the
                        # sebc chain and never blocks the ACT/SP sequencers.
                        scr2 = drp.tile([4 * GB, W], f32r, name="scr2", tag="scr2")
                        nc.gpsimd.dma_start(scr2[0:nb, :], seg[0:nb, :])
                        sebc = bcp.tile([128, 4 * GB * W], bf16, name="sebc", tag="sebc")
                        nc.gpsimd.dma_start(
                            sebc[:, 0:nb * W],
                            scr2[0:nb, :].bitcast(f32)
                            .rearrange("b w -> (b w)").unsqueeze(0)
                            .broadcast_to([128, nb * W]),
                        )

                        def scale_block(gs_, sig_tiles_, sebc_):
                            for gi, g in enumerate(gs_):
                                pi, gl = divmod(gi, 2)
                                sigg_ = sig_tiles_[pi]
                                scaled = scp.tile([128, UC, GBW], bf16,
                                                  name="scaled", tag="scaled")
                                nc.vector.tensor_mul(
                                    scaled[:],
                                    sigg_[:, :, gl, :],
                                    sebc_[:, gi * GBW:(gi + 1) * GBW]
                                    .unsqueeze(1).broadcast_to([128, UC, GBW]),
                                )
                                s4 = scaled[:].rearrange("p u (b w) -> p (u b) w", w=W)
                                tprev = s4
                                wlen = W
                                while wlen > 2:
                                    half = wlen // 2
                                    tn = scp.tile([128, UC * GB, half], bf16,
                                                  name=f"t{half}", tag=f"t{half}")
                                    nc.vector.tensor_max(tn[:], tprev[:, :, 0:half],
                                                         tprev[:, :, half:wlen])
                                    tprev = tn[:]
                                    wlen = half
                                pbf = scp.tile([128, UC * GB, 1], bf16,
                                               name="pbf", tag="pbf")
                                nc.vector.tensor_max(pbf[:], tprev[:, :, 0:1],
                                                     tprev[:, :, 1:2])
                                nc.vector.tensor_copy(
                                    pooledT[:, :, g * GB:(g + 1) * GB],
                                    pbf[:].rearrange("p (u b) one -> p u (b one)",
                                                     u=UC),
                                )

                        pending_scale.append(
                            lambda gs_=gs, st_=sig_tiles, sb_=sebc:
                            scale_block(gs_, st_, sb_))
                        # scale runs 2 blocks behind early (hides the SE
                        # round-trip), 1 block behind near the end (so pooled
                        # finishes promptly for the LSTM chunks)
                        depth = 2 if bi < 7 else 1
                        while len(pending_scale) > depth:
                            pending_scale.pop(0)()

                        for emit in emit_after.get(bi, []):
                            emit()

                    for ps_fn in pending_scale:
                        ps_fn()
                    for emit in emit_after["flush"]:
                        emit()

                    nc.sync.dma_start(d_out, outsb[:])

    nc.compile()
    return nc


def _prep_weights(i):
    """Host-side packing of the replicated (non-batch) tensors."""
    import ml_dtypes

    f8 = ml_dtypes.float8_e4m3
    bf = ml_dtypes.bfloat16

    def f32(a):
        return np.ascontiguousarray(a, dtype=np.float32)

    out = {}
    cwT = f32(i["conv_w"]).T * 32.0                                # [D, U]
    out["cw"] = np.ascontiguousarray(
        cwT.reshape(DC, 128, U).transpose(1, 0, 2).reshape(128, DC * U)
    ).astype(f8)
    out["cb"] = f32(i["conv_b"].reshape(UC, 128).T)
    out["onesm"] = np.full((128, 1), 1.0 / U, bf)
    out["ones32"] = np.ones((1, 4 * GB), np.float32)
    out["onesc"] = np.ones((1, BS), bf)
    out["sewt"] = f32(i["se_w"].T)
    out["seb"] = f32(np.asarray(i["se_b"]).reshape(1, W))
    igo = np.r_[0:512, 1024:2048]  # drop dead forget gate
    # LSTM weights ship as fp8e4m3 scaled by 32 (all values normal-range);
    # gate activations de-scale with scale=1/32. Biases ride the same x32.
    for s, tag in (("f", "l0f"), ("r", "l0r")):
        wT = f32(i[f"w_ih_{tag}"]).T[:, igo] * 32.0                # [512, 1536]
        out[f"w0{s}"] = np.ascontiguousarray(
            wT.reshape(4, 128, 1536).transpose(1, 0, 2).reshape(128, 4 * 1536)
        ).astype(f8)
        bs = (f32(i[f"b_ih_{tag}"]) + f32(i[f"b_hh_{tag}"]))[igo] * 32.0
        out[f"bv0{s}"] = bs.reshape(1, 1536).astype(bf)
    for s, tag in (("f", "l1f"), ("r", "l1r")):
        wT = f32(i[f"w_ih_{tag}"]).T[:, igo] * 32.0                # [1024, 1536]
        out[f"w1{s}"] = np.ascontiguousarray(
            wT.reshape(8, 128, 1536).transpose(1, 0, 2).reshape(128, 8 * 1536)
        ).astype(f8)
        bs = (f32(i[f"b_ih_{tag}"]) + f32(i[f"b_hh_{tag}"]))[igo] * 32.0
        out[f"bv1{s}"] = bs.reshape(1, 1536).astype(bf)
    out["clsw"] = f32(i["cls_w"].reshape(2 * H)).reshape(8, 128).T.copy().astype(bf)
    out["clsb"] = f32(i["cls_b"]).reshape(1, 1)
    return out


def _get_nc():
    global _STATE
    if _STATE is None:
        _STATE = _build_bass()
    return _STATE


def make_in_maps(**inputs):
    import ml_dtypes

    w = _prep_weights(inputs)
    xt = np.ascontiguousarray(
        np.asarray(inputs["x"], dtype=np.float32).transpose(2, 0, 1)
    ).astype(ml_dtypes.float8_e4m3)  # [D, B, W]
    maps = []
    for c in range(NC):
        m = dict(w)
        m["xt"] = np.ascontiguousarray(xt[:, c * BS:(c + 1) * BS, :])
        maps.append(m)
    return maps


def kernel(**inputs):
    nc = _get_nc()
    maps = make_in_maps(**inputs)
    res = run_bass_kernel_spmd(nc, maps, core_ids=list(range(NC)))
    out = np.empty((B, 1), np.float32)
    for c in range(NC):
        out[c * BS:(c + 1) * BS, 0] = res.results[c]["out"][0]
    return out


# revision 44
# speedup vs baseline: 46.9543x; 45.1089x over previous
"""CNN+SE+LSTM fused Trainium2 kernel (v2).

Data-parallel over batch: B=2048 split across 8 NeuronCores (256 each).

Key techniques vs v1:
  - conv1x1 runs in fp8e4m3 with MatmulPerfMode.DoubleRow (2 packed K
    values/cell): 2x PE throughput and 4x less x DMA. conv_w is scaled by
    32 host-side so all weights are fp8-normal; 1/32 folds into the
    sigmoid's scale operand.
  - sigmoid is applied per (uc, group-pair) on a 2-bank PSUM tile
    ([128, 2, 512]) to amortize ACT fixed overhead; channel-mean rows for
    the 4 groups of an SE block land at PSUM partitions 0/32/64/96 of one
    bank (PE column tiling) so one DMA ships the whole block's avg.
  - SE softmax avoids the Exp activation table entirely (Sigmoid and Tanh
    share an ACT table set, Exp does not): e^z = (1+tanh(z/2))/(1-tanh(z/2)),
    with DVE reciprocal_approx_fast. Zero LoadActFuncSet swaps mid-kernel.
  - maxpool-over-window runs as a binary tensor_tensor(max) tree on the DVE
    in bf16 (2x mode), ~1.6x faster than the 1x-mode tensor_reduce.
  - the 2-layer bidirectional LSTM is processed in 3 batch chunks
    (128/96/32 cols) interleaved into the conv/SE stream so its matmuls fill
    PE idle time and the tail after the last pooled group is short. Gate
    biases are added via tiny k=1 bias-matmuls so gate activations can be
    PSUM-func-grouped ([128, 4, cols] per i/g/o); gate element-wise products
    run on GpSimd (Pool) to keep the DVE free.
"""

import numpy as np

import concourse.bass as bass
import concourse.tile as tile
from concourse import bacc, mybir
from concourse.bass_utils import run_bass_kernel_spmd

B, W, D, U, H = 2048, 64, 512, 512, 512
NC = 8
BS = B // NC          # 256 batch rows per core
GB = 8                # batches per group (8 * W = 512 matmul columns)
NG = BS // GB         # 32 groups
GBW = GB * W
BLOCKS = [2, 2, 4, 4, 4, 4, 4, 4, 2, 2]   # SE batching; cum 2,4,8..28,30,32
assert sum(BLOCKS) == NG
DC = D // 128         # 4 contraction chunks
UC = U // 128         # 4 output-channel chunks
# LSTM batch chunks in groups: [start_g, end_g)
CHUNKS = [(0, 16), (16, 28), (28, 32)]

dt = mybir.dt
AF = mybir.ActivationFunctionType
ALU = mybir.AluOpType
AX = mybir.AxisListType
DR = mybir.MatmulPerfMode.DoubleRow

_STATE = None


def _build_bass(unroll=1):
    nc = bacc.Bacc("TRN2", target_bir_lowering=False, debug=False,
                   num_devices=NC, num_swdge_queues=4)

    f32, f32r, bf16, f8 = dt.float32, dt.float32r, dt.bfloat16, dt.float8e4

    d_xt = nc.dram_tensor("xt", [D, BS, W], f8, kind="ExternalInput").ap()
    d_cw = nc.dram_tensor("cw", [128, DC * U], f8, kind="ExternalInput").ap()
    d_cb = nc.dram_tensor("cb", [128, UC], f32, kind="ExternalInput").ap()
    d_onesm = nc.dram_tensor("onesm", [128, 1], bf16, kind="ExternalInput").ap()
    d_ones32 = nc.dram_tensor("ones32", [1, 4 * GB], f32r, kind="ExternalInput").ap()
    d_onesc = nc.dram_tensor("onesc", [1, BS], bf16, kind="ExternalInput").ap()
    d_sewt = nc.dram_tensor("sewt", [W, W], f32r, kind="ExternalInput").ap()
    d_seb = nc.dram_tensor("seb", [1, W], f32r, kind="ExternalInput").ap()
    d_w0, d_bv0, d_w1, d_bv1 = {}, {}, {}, {}
    for s in ("f", "r"):
        d_w0[s] = nc.dram_tensor(f"w0{s}", [128, 4 * 1536], f8, kind="ExternalInput").ap()
        d_bv0[s] = nc.dram_tensor(f"bv0{s}", [1, 1536], bf16, kind="ExternalInput").ap()
        d_w1[s] = nc.dram_tensor(f"w1{s}", [128, 8 * 1536], f8, kind="ExternalInput").ap()
        d_bv1[s] = nc.dram_tensor(f"bv1{s}", [1, 1536], bf16, kind="ExternalInput").ap()
    d_clsw = nc.dram_tensor("clsw", [128, 8], bf16, kind="ExternalInput").ap()
    d_clsb = nc.dram_tensor("clsb", [1, 1], f32, kind="ExternalInput").ap()
    d_out = nc.dram_tensor("out", [1, BS], f32, kind="ExternalOutput").ap()

    with tile.TileContext(nc) as tc:
        with tc.tile_pool(name="wpool", bufs=1) as wpool, \
             tc.tile_pool(name="persist", bufs=1) as persist:
            # static weights, staged up front on the SWDGE path
            cw_t = wpool.tile([128, DC * U], f8, name="cw_t")
            nc.sync.dma_start(cw_t[:], d_cw)
            cb_t = wpool.tile([128, UC], f32, name="cb_t")
            nc.sync.dma_start(cb_t[:], d_cb)
            onesm_t = wpool.tile([128, 1], bf16, name="onesm_t")
            nc.sync.dma_start(onesm_t[:], d_onesm)
            ones32_t = wpool.tile([1, 4 * GB], f32r, name="ones32_t")
            nc.sync.dma_start(ones32_t[:], d_ones32)
            onesc_t = wpool.tile([1, BS], bf16, name="onesc_t")
            nc.sync.dma_start(onesc_t[:], d_onesc)
            sewt_t = wpool.tile([W, W], f32r, name="sewt_t")
            nc.sync.dma_start(sewt_t[:], d_sewt)
            seb_t = wpool.tile([1, W], f32r, name="seb_t")
            nc.sync.dma_start(seb_t[:], d_seb)
            # LSTM weight tiles are allocated here but their (re)loads are
            # issued inside each rep at block 1/3 so the startup DMA slots
            # belong to the x loads.
            w0_t, bv0_t, w1_t, bv1_t = {}, {}, {}, {}
            for s in ("f", "r"):
                w0_t[s] = wpool.tile([128, 4 * 1536], f8, name=f"w0{s}_t")
                bv0_t[s] = wpool.tile([1, 1536], bf16, name=f"bv0{s}_t")
                w1_t[s] = wpool.tile([128, 8 * 1536], f8, name=f"w1{s}_t")
                bv1_t[s] = wpool.tile([1, 1536], bf16, name=f"bv1{s}_t")
            clsw_t = wpool.tile([128, 8], bf16, name="clsw_t")
            nc.sync.dma_start(clsw_t[:], d_clsw)
            clsb_t = wpool.tile([1, 1], f32, name="clsb_t")
            nc.sync.dma_start(clsb_t[:], d_clsb)

            cw_r = cw_t[:].rearrange("p (dc u) -> p dc u", dc=DC)

            pooledT = persist.tile([128, UC, BS], bf16, name="pooledT")
            o0T = persist.tile([128, 8, BS], bf16, name="o0T")
            outsb = persist.tile([1, BS], f32, name="outsb")

            for _rep in range(unroll):
                with tc.tile_pool(name="xp", bufs=4) as xp, \
                     tc.tile_pool(name="sigp", bufs=8) as sigp, \
                     tc.tile_pool(name="scp", bufs=3) as scp, \
                     tc.tile_pool(name="bcp", bufs=3) as bcp, \
                     tc.tile_pool(name="sep", bufs=3) as sep, \
                     tc.tile_pool(name="lp", bufs=2) as lp, \
                     tc.tile_pool(name="drp", bufs=4, space="DRAM") as drp, \
                     tc.tile_pool(name="pps", bufs=2, space="PSUM") as pps:
                    # PSUM budget (8 banks): tag "big" 2x4KB shared by conv cp
                    # and lstm gp (they alternate in time), "us" 2x2KB mean
                    # rows, "small" 2x2KB shared by SE lg and cls psum.

                    # ---------- LSTM emit helpers (interleaved) ----------
                    def lstm_layer(w_t, bv_t, kcs, rhs_fn, cg0, cg1, out_sl,
                                   out_tanh):
                        c0 = cg0 * GB
                        cols = (cg1 - cg0) * GB
                        gates = {}
                        for fi, func in ((0, AF.Sigmoid), (1, AF.Tanh),
                                         (2, AF.Sigmoid)):
                            gp = pps.tile([128, 4, cols], f32, name="gp", tag="big")
                            for q in range(4):
                                m = fi * 4 + q
                                for kc in range(kcs):
                                    nc.tensor.matmul(
                                        gp[:, q, :],
                                        w_t[:, kc * 1536 + m * 128:
                                            kc * 1536 + (m + 1) * 128],
                                        rhs_fn(kc),
                                        start=(kc == 0), stop=False,
                                    )
                                nc.tensor.matmul(
                                    gp[:, q, :],
                                    bv_t[0:1, m * 128:(m + 1) * 128],
                                    onesc_t[0:1, 0:cols],
                                    start=False, stop=True,
                                )
                            gg = lp.tile([128, 4, cols], bf16, name="gg",
                                         tag=f"g{fi}")
                            nc.scalar.activation(gg[:], gp[:], func,
                                                 scale=1.0 / 32.0)
                            gates[fi] = gg
                        cpre = lp.tile([128, 4, cols], bf16, name="cpre", tag="cpre")
                        nc.gpsimd.tensor_mul(cpre[:], gates[0][:], gates[1][:])
                        tcl = lp.tile([128, 4, cols], bf16, name="tcl", tag="tcl")
                        nc.scalar.activation(tcl[:], cpre[:], AF.Tanh)
                        if out_tanh:
                            h = lp.tile([128, 4, cols], bf16, name="h", tag="h")
                            nc.gpsimd.tensor_mul(h[:], gates[2][:], tcl[:])
                            nc.scalar.activation(out_sl, h[:], AF.Tanh)
                        else:
                            nc.gpsimd.tensor_mul(out_sl, gates[2][:], tcl[:])

                    o1c = {}

                    def emit_l0(ci):
                        cg0, cg1 = CHUNKS[ci]
                        c0 = cg0 * GB
                        cols = (cg1 - cg0) * GB
                        for si, s in enumerate(("f", "r")):
                            lstm_layer(
                                w0_t[s], bv0_t[s], 4,
                                lambda kc: pooledT[:, kc, c0:c0 + cols],
                                cg0, cg1,
                                o0T[:, 4 * si:4 * si + 4, c0:c0 + cols],
                                False,
                            )

                    def emit_l1(ci):
                        cg0, cg1 = CHUNKS[ci]
                        c0 = cg0 * GB
                        cols = (cg1 - cg0) * GB
                        oc = lp.tile([128, 8, cols], bf16, name="o1c", tag="o1c")
                        o1c[ci] = oc
                        for si, s in enumerate(("f", "r")):
                            lstm_layer(
                                w1_t[s], bv1_t[s], 8,
                                lambda kc: o0T[:, kc, c0:c0 + cols],
                                cg0, cg1,
                                oc[:, 4 * si:4 * si + 4, :],
                                True,
                            )

                    def emit_cls(ci):
                        cg0, cg1 = CHUNKS[ci]
                        c0 = cg0 * GB
                        cols = (cg1 - cg0) * GB
                        oc = o1c[ci]
                        clsp = pps.tile([1, cols], f32, name="clsp", tag="small")
                        for kc in range(8):
                            nc.tensor.matmul(
                                clsp[:], clsw_t[:, kc:kc + 1], oc[:, kc, :],
                                start=(kc == 0), stop=(kc == 7),
                            )
                        nc.scalar.activation(
                            outsb[0:1, c0:c0 + cols], clsp[:], AF.Tanh,
                            bias=clsb_t[0:1, 0:1], scale=1.0,
                        )

                    # keys are block indices; pooled for blocks <= b-1 is
                    # complete after block b's pending_scale emission.
                    emit_after = {
                        6: [lambda: emit_l0(0)],          # pooled g0..15 (b0-4)
                        7: [lambda: emit_l1(0), lambda: emit_cls(0)],
                        9: [lambda: emit_l0(1)],          # pooled g16..27 (b5-7)
                        "flush": [lambda: emit_l1(1), lambda: emit_cls(1),
                                  lambda: emit_l0(2), lambda: emit_l1(2),
                                  lambda: emit_cls(2)],
                    }

                    # ---------- conv + SE + maxpool stream ----------
                    # scale/maxpool for block b-1 is emitted during block b so
                    # the DVE has work while block b's SE round-trip resolves.
                    g0 = 0
                    pending_scale = []
                    pending_sedma = []
                    for bi, nblk in enumerate(BLOCKS):
                        if bi == 4:
                            # wait_until keeps the scheduler from hoisting
                            # these dep-free loads into the startup DMA burst;
                            # chunked so x loads interleave between slices.
                            # SP ring: no waits, so no head-of-line risk.
                            for ci_, s in enumerate(("f", "r")):
                                for kc in range(2):
                                    with tc.tile_wait_until(0.020 + 0.003 * (2 * ci_ + kc)):
                                        nc.sync.dma_start(
                                            w0_t[s][:, kc * 3072:(kc + 1) * 3072],
                                            d_w0[s][:, kc * 3072:(kc + 1) * 3072])
                                with tc.tile_wait_until(0.028):
                                    nc.sync.dma_start(bv0_t[s][:], d_bv0[s])
                        elif bi == 5:
                            for ci_, s in enumerate(("f", "r")):
                                for kc in range(4):
                                    with tc.tile_wait_until(0.034 + 0.003 * (4 * ci_ + kc)):
                                        nc.sync.dma_start(
                                            w1_t[s][:, kc * 3072:(kc + 1) * 3072],
                                            d_w1[s][:, kc * 3072:(kc + 1) * 3072])
                                with tc.tile_wait_until(0.044):
                                    nc.sync.dma_start(bv1_t[s][:], d_bv1[s])
                        gs = list(range(g0, g0 + nblk))
                        g0 += nblk
                        nb = nblk * GB
                        # previous block's SE bounce DMAs: their seg wait has
                        # resolved by now, so they don't stall the SP queue
                        while pending_sedma:
                            pending_sedma.pop(0)()
                        scr1 = drp.tile([4, GBW], f32r, name="scr1", tag="scr1")
                        sig_tiles = []
                        for ps_ in range(0, nblk, 2):
                            gpair = gs[ps_:ps_ + 2]
                            sigg = sigp.tile([128, UC, 2, GBW], bf16,
                                             name="sigg", tag="sig")
                            sig_tiles.append(sigg)
                            xts = []
                            for g in gpair:
                                xt = xp.tile([128, DC, GBW], f8, name="xt", tag="x")
                                nc.sync.dma_start(
                                    xt[:],
                                    d_xt[:, g * GB:(g + 1) * GB, :].rearrange(
                                        "(dc p) b w -> p dc (b w)", p=128),
                                )
                                xts.append(xt)
                            for uc in range(UC):
                                cp = pps.tile([128, 2, GBW], f32, name="cp", tag="big")
                                for gl in range(2):
                                    for kp in range(2):
                                        nc.tensor.matmul(
                                            cp[:, gl, :],
                                            cw_r[:, 2 * kp:2 * kp + 2,
                                                 uc * 128:(uc + 1) * 128],
                                            xts[gl][:, 2 * kp:2 * kp + 2, :],
                                            start=(kp == 0), stop=(kp == 1),
                                            perf_mode=DR,
                                        )
                                nc.scalar.activation(
                                    sigg[:, uc, :, :], cp[:], AF.Sigmoid,
                                    bias=cb_t[:, uc:uc + 1], scale=1.0 / 32.0,
                                )
                            # channel-mean rows for the pair at PSUM
                            # partitions 0 and 32 of a 1-bank tile
                            us2 = pps.tile([128, GBW], f32, name="us2", tag="us")
                            for gl, g in enumerate(gpair):
                                for uc in range(UC):
                                    nc.tensor.matmul(
                                        us2[32 * gl:32 * gl + 1, :],
                                        onesm_t[:],
                                        sigg[:, uc, gl, :],
                                        start=(uc == 0), stop=(uc == UC - 1),
                                    )
                            # Neither DMA nor GpSimd can read PSUM: bounce via
                            # a copy (alternating ACT/DVE to share the cost),
                            # then transpose straight into avgT (SBUF->SBUF)
                            # engines need partition step 1, so copy the whole
                            # 0..32 lane range (junk lanes 1..31 cost nothing:
                            # engine time scales with free size only), then
                            # bounce the two rows through DRAM for the
                            # w-transpose (DRAM APs have no partition rules)
                            avg2 = sep.tile([33, GBW], f32r, name="avg2", tag="avg2")
                            if (bi + ps_) % 2 == 0:
                                nc.scalar.copy(avg2[0:33, :], us2[0:33, :])
                            else:
                                nc.vector.tensor_copy(avg2[0:33, :], us2[0:33, :])
                            for gl in range(2):
                                nc.sync.dma_start(
                                    scr1[ps_ + gl:ps_ + gl + 1, :],
                                    avg2[32 * gl:32 * gl + 1, :],
                                )

                        # ---- SE for the block ----
                        avgT = sep.tile([W, 4 * GB], f32r, name="avgT", tag="avgT")
                        nc.sync.dma_start(
                            avgT[:, 0:nb],
                            scr1[0:nblk, :].rearrange("g (b w) -> (w) g b", w=W),
                        )
                        lg = pps.tile([4 * GB, W], f32, name="lg", tag="small")
                        nc.tensor.matmul(lg[0:nb, :], avgT[:, 0:nb], sewt_t[:],
                                         start=True, stop=False)
                        nc.tensor.matmul(lg[0:nb, :], ones32_t[:, 0:nb], seb_t[:],
                                         start=False, stop=True)
                        # softmax via tanh: e^z = (1+tanh(z/2))/(1-tanh(z/2))
                        th = sep.tile([4 * GB, W], f32, name="th", tag="th")
                        nc.scalar.activation(th[0:nb, :], lg[0:nb, :], AF.Tanh,
                                             scale=0.5)
                        den = sep.tile([4 * GB, W], f32, name="den", tag="den")
                        nc.vector.tensor_scalar(den[0:nb, :], th[0:nb, :],
                                                -1.0, 1.0, ALU.mult, ALU.add)
                        rden = sep.tile([4 * GB, W], f32, name="rden", tag="rden")
                        nc.vector.reciprocal_approx_fast(rden[0:nb, :], den[0:nb, :])
                        num = sep.tile([4 * GB, W], f32, name="num", tag="num")
                        nc.vector.tensor_scalar_add(num[0:nb, :], th[0:nb, :], 1.0)
                        E = sep.tile([4 * GB, W], f32, name="E", tag="E")
                        nc.vector.tensor_mul(E[0:nb, :], num[0:nb, :], rden[0:nb, :])
                        S = sep.tile([4 * GB, 1], f32, name="S", tag="S")
                        nc.vector.reduce_sum(S[0:nb, :], E[0:nb, :], axis=AX.X)
                        R = sep.tile([4 * GB, 1], f32, name="R", tag="R")
                        nc.vector.reciprocal_approx_fast(R[0:nb, :], S[0:nb, :])
                        seg = sep.tile([4 * GB, W], bf16, name="seg", tag="seg")
                        nc.vector.tensor_scalar_mul(seg[0:nb, :], E[0:nb, :],
                                                    R[0:nb, 0:1])
                        # SE broadcast bounce rides the SP HWDGE ring (SWDGE
                        # round-trips measure ~40-60us on HW). Emission is
                        # deferred to the next block's section so the seg wait
                        # never stalls the SP queue ahead of the x loads.
                        scr2 = drp.tile([4 * GB, W], bf16, name="scr2", tag="scr2")
                        sebc = bcp.tile([128, 4 * GB * W], bf16, name="sebc", tag="sebc")

                        def se_dma(nb_=nb, seg_=seg, scr2_=scr2, sebc_=sebc):
                            nc.sync.dma_start(scr2_[0:nb_, :], seg_[0:nb_, :])
                            nc.sync.dma_start(
                                sebc_[:, 0:nb_ * W],
                                scr2_[0:nb_, :]
                                .rearrange("b w -> (b w)").unsqueeze(0)
                                .broadcast_to([128, nb_ * W]),
                            )
                        pending_sedma.append(se_dma)

                        def scale_block(gs_, sig_tiles_, sebc_):
                            for gi, g in enumerate(gs_):
                                pi, gl = divmod(gi, 2)
                                sigg_ = sig_tiles_[pi]
                                scaled = scp.tile([128, UC, GBW], bf16,
                                                  name="scaled", tag="scaled")
                                nc.vector.tensor_mul(
                                    scaled[:],
                                    sigg_[:, :, gl, :],
                                    sebc_[:, gi * GBW:(gi + 1) * GBW]
                                    .unsqueeze(1).broadcast_to([128, UC, GBW]),
                                )
                                s4 = scaled[:].rearrange("p u (b w) -> p (u b) w", w=W)
                                tprev = s4
                                wlen = W
                                while wlen > 2:
                                    half = wlen // 2
                                    tn = scp.tile([128, UC * GB, half], bf16,
                                                  name=f"t{half}", tag=f"t{half}")
                                    nc.vector.tensor_max(tn[:], tprev[:, :, 0:half],
                                                         tprev[:, :, half:wlen])
                                    tprev = tn[:]
                                    wlen = half
                                pbf = scp.tile([128, UC * GB, 1], bf16,
                                               name="pbf", tag="pbf")
                                nc.vector.tensor_max(pbf[:], tprev[:, :, 0:1],
                                                     tprev[:, :, 1:2])
                                nc.vector.tensor_copy(
                                    pooledT[:, :, g * GB:(g + 1) * GB],
                                    pbf[:].rearrange("p (u b) one -> p u (b one)",
                                                     u=UC),
                                )

                        pending_scale.append(
                            lambda gs_=gs, st_=sig_tiles, sb_=sebc:
                            scale_block(gs_, st_, sb_))
                        # scale runs 2 blocks behind early (hides the SE
                        # round-trip), 1 block behind near the end (so pooled
                        # finishes promptly for the LSTM chunks)
                        depth = 2 if bi < 7 else 1
                        while len(pending_scale) > depth:
                            pending_scale.pop(0)()

                        for emit in emit_after.get(bi, []):
                            emit()

                    while pending_sedma:
                        pending_sedma.pop(0)()
                    for ps_fn in pending_scale:
                        ps_fn()
                    for emit in emit_after["flush"]:
                        emit()

                    nc.sync.dma_start(d_out, outsb[:])

    nc.compile()
    return nc


def _prep_weights(i):
    """Host-side packing of the replicated (non-batch) tensors."""
    import ml_dtypes

    f8 = ml_dtypes.float8_e4m3
    bf = ml_dtypes.bfloat16

    def f32(a):
        return np.ascontiguousarray(a, dtype=np.float32)

    out = {}
    cwT = f32(i["conv_w"]).T * 32.0                                # [D, U]
    out["cw"] = np.ascontiguousarray(
        cwT.reshape(DC, 128, U).transpose(1, 0, 2).reshape(128, DC * U)
    ).astype(f8)
    out["cb"] = f32(i["conv_b"].reshape(UC, 128).T)
    out["onesm"] = np.full((128, 1), 1.0 / U, bf)
    out["ones32"] = np.ones((1, 4 * GB), np.float32)
    out["onesc"] = np.ones((1, BS), bf)
    out["sewt"] = f32(i["se_w"].T)
    out["seb"] = f32(np.asarray(i["se_b"]).reshape(1, W))
    igo = np.r_[0:512, 1024:2048]  # drop dead forget gate
    # LSTM weights ship as fp8e4m3 scaled by 32 (all values normal-range);
    # gate activations de-scale with scale=1/32. Biases ride the same x32.
    for s, tag in (("f", "l0f"), ("r", "l0r")):
        wT = f32(i[f"w_ih_{tag}"]).T[:, igo] * 32.0                # [512, 1536]
        out[f"w0{s}"] = np.ascontiguousarray(
            wT.reshape(4, 128, 1536).transpose(1, 0, 2).reshape(128, 4 * 1536)
        ).astype(f8)
        bs = (f32(i[f"b_ih_{tag}"]) + f32(i[f"b_hh_{tag}"]))[igo] * 32.0
        out[f"bv0{s}"] = bs.reshape(1, 1536).astype(bf)
    for s, tag in (("f", "l1f"), ("r", "l1r")):
        wT = f32(i[f"w_ih_{tag}"]).T[:, igo] * 32.0                # [1024, 1536]
        out[f"w1{s}"] = np.ascontiguousarray(
            wT.reshape(8, 128, 1536).transpose(1, 0, 2).reshape(128, 8 * 1536)
        ).astype(f8)
        bs = (f32(i[f"b_ih_{tag}"]) + f32(i[f"b_hh_{tag}"]))[igo] * 32.0
        out[f"bv1{s}"] = bs.reshape(1, 1536).astype(bf)
    out["clsw"] = f32(i["cls_w"].reshape(2 * H)).reshape(8, 128).T.copy().astype(bf)
    out["clsb"] = f32(i["cls_b"]).reshape(1, 1)
    return out


def _get_nc():
    global _STATE
    if _STATE is None:
        _STATE = _build_bass()
    return _STATE


def make_in_maps(**inputs):
    import ml_dtypes

    w = _prep_weights(inputs)
    xt = np.ascontiguousarray(
        np.asarray(inputs["x"], dtype=np.float32).transpose(2, 0, 1)
    ).astype(ml_dtypes.float8_e4m3)  # [D, B, W]
    maps = []
    for c in range(NC):
        m = dict(w)
        m["xt"] = np.ascontiguousarray(xt[:, c * BS:(c + 1) * BS, :])
        maps.append(m)
    return maps


def kernel(**inputs):
    nc = _get_nc()
    maps = make_in_maps(**inputs)
    res = run_bass_kernel_spmd(nc, maps, core_ids=list(range(NC)))
    out = np.empty((B, 1), np.float32)
    for c in range(NC):
        out[c * BS:(c + 1) * BS, 0] = res.results[c]["out"][0]
    return out
